# revision 28
# baseline (speedup 1.0000x reference)
"""CrossCosineEmbeddingLoss on 8 Trainium2 NeuronCores.

reference:
    xn = x / max(||x_i||, 1e-8);  yn = y / max(||y_j||, 1e-8)
    S = xn @ yn.T                         # [8192, 8192]
    loss = (sum_{i!=j} relu(S_ij) + sum_i (1 - S_ii)) / 8192^2

Sharding: rows of x across the 8 cores (1024 rows each); y replicated.
Each core computes its [1024, 8192] slab of S in bf16 on the PE array
(fp32 PSUM accumulation), relu+row-sums it on DVE/ACT during PSUM
eviction, and separately computes its 1024 diagonal terms exactly in
fp32.  The host folds the per-core partial sums in float64:

    loss = sum_c [ main_c + 1024 - diag_c ] / 8192^2

where main_c = sum relu(S_slab) (incl. the slab's diagonal entries) and
diag_c = sum_i (S_ii + relu(S_ii)), so the relu(S_ii) the main sum
picked up is removed and replaced by (1 - S_ii).

bf16 matmul error on the final scalar measured at ~3e-6 relative.
"""

import os
from contextlib import ExitStack

import numpy as np

import concourse.bass as bass
import concourse.tile as tile
from concourse import bacc, bass_utils, masks, mybir

dt = mybir.dt
Alu = mybir.AluOpType
Act = mybir.ActivationFunctionType

N = 8192          # rows of x and y
D = 1024          # feature dim
NCORES = 8
M = N // NCORES   # 1024 rows of x per core
P = 128           # partitions
KCH = D // P      # 8 contraction chunks
XT = M // P       # 8 x row-tiles per core
GROUPS = 8        # y processed in column groups
GROWS = N // GROUPS       # 2048 y rows per group
GT = GROWS // P           # 16 y row-tiles per group
JBW = 512                 # j-block width (one PSUM bank of fp32)
JB = GROWS // JBW         # 4 j-blocks per group
NJBLK = N // JBW          # 16 j-blocks total
OUT_COLS = XT * NJBLK + XT  # 128 main cols + 8 diag cols = 136

EPS = 1e-8

# "pe":   transpose via tensor engine + PSUM + copy
# "dram": normalized bf16 tiles round-trip through DRAM, transposed on the
#         way back with one big DMA-transpose per (group, k-chunk)
TRANSPOSE_MODE = os.environ.get("CCEL_TRANSPOSE", "dram")
SKIP_DIAG = os.environ.get("CCEL_SKIP_DIAG", "0") == "1"
SKIP_MM = os.environ.get("CCEL_SKIP_MM", "0") == "1"
SKIP_Y = os.environ.get("CCEL_SKIP_Y", "0") == "1"


_SSQ_FLIP = [0]


def _row_norm_recip(nc, sqpool, smpool, src, nbpool=None):
    """r[128,1] = 1 / max(sqrt(sum(src^2, axis=free)), EPS), fp32.

    Alternates the square+row-sum between ACT and DVE to balance load.
    """
    ssq = smpool.tile([P, 1], dt.float32, tag="ssq", name="ssq")
    sqg = sqpool.tile([P, D], dt.bfloat16, tag="sqg", name="sqg")
    _SSQ_FLIP[0] ^= 1
    if _SSQ_FLIP[0] or nbpool is None:
        nc.scalar.activation(sqg[:], src[:], Act.Square, accum_out=ssq[:])
    else:
        sqf = nbpool.tile([P, D], dt.float32, tag="prod", name="sqf")
        nc.vector.tensor_mul(sqf[:], src[:], src[:])
        nc.vector.tensor_scalar(
            out=sqg[:], in0=sqf[:], scalar1=0.0, scalar2=None,
            op0=Alu.add, op1=Alu.add, accum_out=ssq[:])
    nrm = smpool.tile([P, 1], dt.float32, tag="nrm", name="nrm")
    nc.scalar.sqrt(nrm[:], ssq[:])
    nc.vector.tensor_scalar_max(nrm[:], nrm[:], EPS)
    r = smpool.tile([P, 1], dt.float32, tag="rrec", name="rrec")
    nc.vector.reciprocal(r[:], nrm[:])
    return r


def kernel_body(ctx: ExitStack, tc: "tile.TileContext", out_ap, x_ap, y_ap, yd_ap):
    nc = tc.nc

    cpool = ctx.enter_context(tc.tile_pool(name="const", bufs=1))
    colsums = cpool.tile([P, OUT_COLS], dt.float32, name="colsums")

    ld = ctx.enter_context(tc.tile_pool(name="ld", bufs=8))
    nb = ctx.enter_context(tc.tile_pool(name="nb", bufs=4))
    sq = ctx.enter_context(tc.tile_pool(name="sq", bufs=4))
    sm = ctx.enter_context(tc.tile_pool(name="sm", bufs=6))
    xnt = ctx.enter_context(tc.tile_pool(name="xnt", bufs=1))
    ynt = ctx.enter_context(tc.tile_pool(name="ynt", bufs=3))
    if TRANSPOSE_MODE == "pe":
        ident = cpool.tile([P, P], dt.bfloat16, name="ident")
        masks.make_identity(nc, ident[:])
        tp = ctx.enter_context(tc.tile_pool(name="tp", bufs=2, space="PSUM"))
        mm = ctx.enter_context(tc.tile_pool(name="mm", bufs=6, space="PSUM"))
        dram = None
    else:
        tp = None
        mm = ctx.enter_context(tc.tile_pool(name="mm", bufs=8, space="PSUM"))
        dram = ctx.enter_context(tc.tile_pool(name="dram", bufs=1, space="DRAM"))

    copy_engines = [nc.vector, nc.scalar]
    copy_idx = 0

    def transpose_128(dst_ap, src_ap):
        """dst[128,128] = src[128,128].T (both SBUF bf16)."""
        nonlocal copy_idx
        if TRANSPOSE_MODE == "dma":
            nc.sync.dma_start(dst_ap, src_ap, transpose=True)
        else:
            pt = tp.tile([P, P], dt.bfloat16, tag="tp", name="pt")
            nc.tensor.transpose(pt[:], src_ap, ident[:])
            if copy_idx % 2 == 0:
                nc.vector.tensor_copy(dst_ap, pt[:])
            else:
                nc.scalar.copy(dst_ap, pt[:])
            copy_idx += 1

    # ---------------- x prologue: xnT[k] = normalized-x^T, bf16 -----------
    xnT = [xnt.tile([P, M], dt.bfloat16, name=f"xnT{k}") for k in range(KCH)]
    if TRANSPOSE_MODE == "dram":
        xnb_dram = dram.tile([M, D], dt.bfloat16, name="xnb_dram")
    for it in range(XT):
        xt = ld.tile([P, D], dt.float32, tag="ld", name="xt")
        nc.sync.dma_start(xt[:], x_ap[it * P:(it + 1) * P, :])
        rx = _row_norm_recip(nc, sq, sm, xt)
        xnb = nb.tile([P, D], dt.bfloat16, tag="nb", name="xnb")
        nc.vector.tensor_scalar_mul(xnb[:], xt[:], rx[:])
        if TRANSPOSE_MODE == "dram":
            nc.sync.dma_start(xnb_dram[it * P:(it + 1) * P, :], xnb[:])
        else:
            for k in range(KCH):
                transpose_128(xnT[k][:, it * P:(it + 1) * P],
                              xnb[:, k * P:(k + 1) * P])

    if TRANSPOSE_MODE == "dram":
        for k in range(KCH):
            nc.sync.dma_start_transpose(
                out=xnT[k][:], in_=xnb_dram[:, k * P:(k + 1) * P])

    # ---------------- y groups: build ynT slab, matmul, relu-reduce -------
    for g in range(GROUPS if not SKIP_Y else 0):
        ynTg = [ynt.tile([P, GROWS], dt.bfloat16, tag=f"ynT{k}", name=f"ynT{k}_g{g}")
                for k in range(KCH)]
        if TRANSPOSE_MODE == "dram":
            ynb_dram = dram.tile([GROWS, D], dt.bfloat16, name=f"ynb_dram{g}")
        for jt in range(GT):
            row0 = g * GROWS + jt * P
            yt = ld.tile([P, D], dt.float32, tag="ld", name="yt")
            nc.sync.dma_start(yt[:], y_ap[row0:row0 + P, :])
            ry = _row_norm_recip(nc, sq, sm, yt)
            ynb = nb.tile([P, D], dt.bfloat16, tag="nb", name="ynb")
            nc.vector.tensor_scalar_mul(ynb[:], yt[:], ry[:])
            if TRANSPOSE_MODE == "dram":
                nc.sync.dma_start(ynb_dram[jt * P:(jt + 1) * P, :], ynb[:])
            else:
                for k in range(KCH):
                    transpose_128(ynTg[k][:, jt * P:(jt + 1) * P],
                                  ynb[:, k * P:(k + 1) * P])
        if TRANSPOSE_MODE == "dram":
            for k in range(KCH):
                nc.sync.dma_start_transpose(
                    out=ynTg[k][:], in_=ynb_dram[:, k * P:(k + 1) * P])

        for i in range(XT if not SKIP_MM else 0):
            psums = [mm.tile([P, JBW], dt.float32, tag="mm", name=f"pm{g}_{i}_{jb}")
                     for jb in range(JB)]
            for k in range(KCH):
                for jb in range(JB):
                    nc.tensor.matmul(
                        psums[jb][:],
                        xnT[k][:, i * P:(i + 1) * P],
                        ynTg[k][:, jb * JBW:(jb + 1) * JBW],
                        start=(k == 0), stop=(k == KCH - 1),
                    )
            for jb in range(JB):
                col = i * NJBLK + g * JB + jb
                acc = colsums[:, col:col + 1]
                if (i * JB + jb) % 2 == 0:
                    nc.vector.tensor_scalar(
                        out=psums[jb][:], in0=psums[jb][:],
                        scalar1=0.0, scalar2=None,
                        op0=Alu.max, op1=Alu.add, accum_out=acc,
                    )
                else:
                    nc.scalar.activation(
                        out=psums[jb][:], in_=psums[jb][:],
                        func=Act.Relu, accum_out=acc,
                    )

    # --------------- exact fp32 diagonal terms (tail, overlaps matmuls) ---
    for it in range(XT if not SKIP_DIAG else 0):
        xt = ld.tile([P, D], dt.float32, tag="ld", name="xt2")
        nc.sync.dma_start(xt[:], x_ap[it * P:(it + 1) * P, :])
        rx = _row_norm_recip(nc, sq, sm, xt)
        ydt = ld.tile([P, D], dt.float32, tag="ld", name="ydt")
        nc.sync.dma_start(ydt[:], yd_ap[it * P:(it + 1) * P, :])
        ryd = _row_norm_recip(nc, sq, sm, ydt)
        xy = sm.tile([P, 1], dt.float32, tag="xy", name="xy")
        prod = nb.tile([P, D], dt.float32, tag="prod", name="prod")
        nc.vector.tensor_mul(prod[:], xt[:], ydt[:])
        tgarb = sq.tile([P, D], dt.bfloat16, tag="sqg", name="tgarb")
        nc.vector.tensor_scalar(
            out=tgarb[:], in0=prod[:], scalar1=0.0, scalar2=None,
            op0=Alu.add, op1=Alu.add, accum_out=xy[:],
        )
        sii = sm.tile([P, 1], dt.float32, tag="sii", name="sii")
        nc.vector.tensor_mul(sii[:], xy[:], rx[:])
        nc.vector.tensor_mul(sii[:], sii[:], ryd[:])
        rel = sm.tile([P, 1], dt.float32, tag="rel", name="rel")
        nc.vector.tensor_scalar_max(rel[:], sii[:], 0.0)
        dcol = XT * NJBLK + it
        nc.vector.tensor_add(colsums[:, dcol:dcol + 1], sii[:], rel[:])

    nc.sync.dma_start(out_ap[:, :], colsums[:])


_PROGRAM_CACHE = {}


def build_program():
    if "nc" in _PROGRAM_CACHE:
        return _PROGRAM_CACHE["nc"]
    nc = bacc.Bacc(
        "TRN2",
        target_bir_lowering=False,
        debug=False,
        enable_asserts=False,
    )
    x_ap = nc.dram_tensor("x", [M, D], dt.float32, kind="ExternalInput").ap()
    y_ap = nc.dram_tensor("y", [N, D], dt.float32, kind="ExternalInput").ap()
    yd_ap = nc.dram_tensor("ydiag", [M, D], dt.float32, kind="ExternalInput").ap()
    out_ap = nc.dram_tensor("out", [P, OUT_COLS], dt.float32, kind="ExternalOutput").ap()

    with tile.TileContext(nc) as tc:
        with ExitStack() as ctx:
            kernel_body(ctx, tc, out_ap, x_ap, y_ap, yd_ap)
    nc.compile()
    _PROGRAM_CACHE["nc"] = nc
    return nc


def make_in_maps(x: np.ndarray, y: np.ndarray):
    x = np.ascontiguousarray(np.asarray(x, dtype=np.float32))
    y = np.ascontiguousarray(np.asarray(y, dtype=np.float32))
    in_maps = []
    for c in range(NCORES):
        sl = slice(c * M, (c + 1) * M)
        in_maps.append({
            "x": x[sl],
            "y": y,
            "ydiag": y[sl],
        })
    return in_maps


def finish(outs: "list[np.ndarray]") -> np.float32:
    """Fold per-core [128, 136] partials into the scalar loss (float64)."""
    total = 0.0
    for o in outs:
        o = np.asarray(o, dtype=np.float64)
        main = o[:, :XT * NJBLK].sum()
        diag = o[:, XT * NJBLK:].sum()
        total += main + float(M) - diag
    return np.float32(total / (float(N) * float(N)))


def kernel(x: np.ndarray, y: np.ndarray):
    nc = build_program()
    in_maps = make_in_maps(x, y)
    res = bass_utils.run_bass_kernel_spmd(nc, in_maps, core_ids=list(range(NCORES)))
    outs = [r["out"] for r in res.results]
    loss = finish(outs)
    kernel.last_results = res
    return {"loss": loss}


kernel.last_results = None


# revision 29
# speedup vs baseline: 1.1213x; 1.1213x over previous
"""CrossCosineEmbeddingLoss on 8 Trainium2 NeuronCores.

reference:
    xn = x / max(||x_i||, 1e-8);  yn = y / max(||y_j||, 1e-8)
    S = xn @ yn.T                         # [8192, 8192]
    loss = (sum_{i!=j} relu(S_ij) + sum_i (1 - S_ii)) / 8192^2

Sharding: rows of x across the 8 cores (1024 rows each); y replicated.
Each core computes its [1024, 8192] slab of S in bf16 on the PE array
(fp32 PSUM accumulation), relu+row-sums it on DVE/ACT during PSUM
eviction, and separately computes its 1024 diagonal terms exactly in
fp32.  The host folds the per-core partial sums in float64:

    loss = sum_c [ main_c + 1024 - diag_c ] / 8192^2

where main_c = sum relu(S_slab) (incl. the slab's diagonal entries) and
diag_c = sum_i (S_ii + relu(S_ii)), so the relu(S_ii) the main sum
picked up is removed and replaced by (1 - S_ii).

bf16 matmul error on the final scalar measured at ~3e-6 relative.
"""

import os
from contextlib import ExitStack

import numpy as np

import concourse.bass as bass
import concourse.tile as tile
from concourse import bacc, bass_utils, masks, mybir

dt = mybir.dt
Alu = mybir.AluOpType
Act = mybir.ActivationFunctionType

N = 8192          # rows of x and y
D = 1024          # feature dim
NCORES = 8
M = N // NCORES   # 1024 rows of x per core
P = 128           # partitions
KCH = D // P      # 8 contraction chunks
XT = M // P       # 8 x row-tiles per core
GROUPS = 8        # y processed in column groups
GROWS = N // GROUPS       # 2048 y rows per group
GT = GROWS // P           # 16 y row-tiles per group
JBW = 512                 # j-block width (one PSUM bank of fp32)
JB = GROWS // JBW         # 4 j-blocks per group
NJBLK = N // JBW          # 16 j-blocks total
OUT_COLS = XT * NJBLK + XT  # 128 main cols + 8 diag cols = 136

EPS = 1e-8

# "pe":   transpose via tensor engine + PSUM + copy
# "dram": normalized bf16 tiles round-trip through DRAM, transposed on the
#         way back with one big DMA-transpose per (group, k-chunk)
TRANSPOSE_MODE = os.environ.get("CCEL_TRANSPOSE", "dram")
SKIP_DIAG = os.environ.get("CCEL_SKIP_DIAG", "0") == "1"
SKIP_MM = os.environ.get("CCEL_SKIP_MM", "0") == "1"
SKIP_Y = os.environ.get("CCEL_SKIP_Y", "0") == "1"


_SSQ_FLIP = [0]


def _row_norm_recip(nc, sqpool, smpool, src, nbpool=None):
    """r[128,1] = 1 / max(sqrt(sum(src^2, axis=free)), EPS), fp32.

    Alternates the square+row-sum between ACT and DVE to balance load.
    """
    ssq = smpool.tile([P, 1], dt.float32, tag="ssq", name="ssq")
    sqg = sqpool.tile([P, D], dt.bfloat16, tag="sqg", name="sqg")
    _SSQ_FLIP[0] ^= 1
    if _SSQ_FLIP[0] or nbpool is None:
        nc.scalar.activation(sqg[:], src[:], Act.Square, accum_out=ssq[:])
    else:
        sqf = nbpool.tile([P, D], dt.float32, tag="prod", name="sqf")
        nc.vector.tensor_mul(sqf[:], src[:], src[:])
        nc.vector.tensor_scalar(
            out=sqg[:], in0=sqf[:], scalar1=0.0, scalar2=None,
            op0=Alu.add, op1=Alu.add, accum_out=ssq[:])
    nrm = smpool.tile([P, 1], dt.float32, tag="nrm", name="nrm")
    nc.scalar.sqrt(nrm[:], ssq[:])
    nc.vector.tensor_scalar_max(nrm[:], nrm[:], EPS)
    r = smpool.tile([P, 1], dt.float32, tag="rrec", name="rrec")
    nc.vector.reciprocal(r[:], nrm[:])
    return r


def kernel_body(ctx: ExitStack, tc: "tile.TileContext", out_ap, x_ap, y_ap, yd_ap):
    nc = tc.nc

    cpool = ctx.enter_context(tc.tile_pool(name="const", bufs=1))
    colsums = cpool.tile([P, OUT_COLS], dt.float32, name="colsums")

    ld = ctx.enter_context(tc.tile_pool(name="ld", bufs=8))
    nb = ctx.enter_context(tc.tile_pool(name="nb", bufs=4))
    sq = ctx.enter_context(tc.tile_pool(name="sq", bufs=4))
    sm = ctx.enter_context(tc.tile_pool(name="sm", bufs=6))
    xnt = ctx.enter_context(tc.tile_pool(name="xnt", bufs=1))
    ynt = ctx.enter_context(tc.tile_pool(name="ynt", bufs=3))
    ident = cpool.tile([P, P], dt.bfloat16, name="ident")
    masks.make_identity(nc, ident[:])
    tp = ctx.enter_context(tc.tile_pool(name="tp", bufs=2, space="PSUM"))
    mm = ctx.enter_context(tc.tile_pool(name="mm", bufs=6, space="PSUM"))
    if TRANSPOSE_MODE == "dram":
        dram = ctx.enter_context(tc.tile_pool(name="dram", bufs=1, space="DRAM"))
    else:
        dram = None

    copy_engines = [nc.vector, nc.scalar]
    copy_idx = 0

    def transpose_128(dst_ap, src_ap):
        """dst[128,128] = src[128,128].T (both SBUF bf16)."""
        nonlocal copy_idx
        if TRANSPOSE_MODE == "dma":
            nc.sync.dma_start(dst_ap, src_ap, transpose=True)
        else:
            pt = tp.tile([P, P], dt.bfloat16, tag="tp", name="pt")
            nc.tensor.transpose(pt[:], src_ap, ident[:])
            if copy_idx % 2 == 0:
                nc.vector.tensor_copy(dst_ap, pt[:])
            else:
                nc.scalar.copy(dst_ap, pt[:])
            copy_idx += 1

    # ---------------- x prologue: xnT[k] = normalized-x^T, bf16 -----------
    xnT = [xnt.tile([P, M], dt.bfloat16, name=f"xnT{k}") for k in range(KCH)]
    for it in range(XT):
        xt = ld.tile([P, D], dt.float32, tag="ld", name="xt")
        nc.sync.dma_start(xt[:], x_ap[it * P:(it + 1) * P, :])
        rx = _row_norm_recip(nc, sq, sm, xt)
        xnb = nb.tile([P, D], dt.bfloat16, tag="nb", name="xnb")
        nc.vector.tensor_scalar_mul(xnb[:], xt[:], rx[:])
        for k in range(KCH):
            transpose_128(xnT[k][:, it * P:(it + 1) * P],
                          xnb[:, k * P:(k + 1) * P])

    # ---------------- y groups: build ynT slab, matmul, relu-reduce -------
    for g in range(GROUPS if not SKIP_Y else 0):
        ynTg = [ynt.tile([P, GROWS], dt.bfloat16, tag=f"ynT{k}", name=f"ynT{k}_g{g}")
                for k in range(KCH)]
        if TRANSPOSE_MODE == "dram":
            ynb_dram = dram.tile([GROWS, D], dt.bfloat16, name=f"ynb_dram{g}")
        for jt in range(GT):
            row0 = g * GROWS + jt * P
            yt = ld.tile([P, D], dt.float32, tag="ld", name="yt")
            nc.sync.dma_start(yt[:], y_ap[row0:row0 + P, :])
            ry = _row_norm_recip(nc, sq, sm, yt)
            ynb = nb.tile([P, D], dt.bfloat16, tag="nb", name="ynb")
            nc.vector.tensor_scalar_mul(ynb[:], yt[:], ry[:])
            if TRANSPOSE_MODE == "dram":
                nc.sync.dma_start(ynb_dram[jt * P:(jt + 1) * P, :], ynb[:])
            else:
                for k in range(KCH):
                    transpose_128(ynTg[k][:, jt * P:(jt + 1) * P],
                                  ynb[:, k * P:(k + 1) * P])
        if TRANSPOSE_MODE == "dram":
            for k in range(KCH):
                nc.sync.dma_start_transpose(
                    out=ynTg[k][:], in_=ynb_dram[:, k * P:(k + 1) * P])

        for i in range(XT if not SKIP_MM else 0):
            psums = [mm.tile([P, JBW], dt.float32, tag="mm", name=f"pm{g}_{i}_{jb}")
                     for jb in range(JB)]
            for k in range(KCH):
                for jb in range(JB):
                    nc.tensor.matmul(
                        psums[jb][:],
                        xnT[k][:, i * P:(i + 1) * P],
                        ynTg[k][:, jb * JBW:(jb + 1) * JBW],
                        start=(k == 0), stop=(k == KCH - 1),
                    )
            for jb in range(JB):
                col = i * NJBLK + g * JB + jb
                acc = colsums[:, col:col + 1]
                if (i * JB + jb) % 2 == 0:
                    nc.vector.tensor_scalar(
                        out=psums[jb][:], in0=psums[jb][:],
                        scalar1=0.0, scalar2=None,
                        op0=Alu.max, op1=Alu.add, accum_out=acc,
                    )
                else:
                    nc.scalar.activation(
                        out=psums[jb][:], in_=psums[jb][:],
                        func=Act.Relu, accum_out=acc,
                    )

    # --------------- exact fp32 diagonal terms (tail, overlaps matmuls) ---
    for it in range(XT if not SKIP_DIAG else 0):
        xt = ld.tile([P, D], dt.float32, tag="ld", name="xt2")
        nc.sync.dma_start(xt[:], x_ap[it * P:(it + 1) * P, :])
        rx = _row_norm_recip(nc, sq, sm, xt)
        ydt = ld.tile([P, D], dt.float32, tag="ld", name="ydt")
        nc.sync.dma_start(ydt[:], yd_ap[it * P:(it + 1) * P, :])
        ryd = _row_norm_recip(nc, sq, sm, ydt)
        xy = sm.tile([P, 1], dt.float32, tag="xy", name="xy")
        prod = nb.tile([P, D], dt.float32, tag="prod", name="prod")
        nc.vector.tensor_mul(prod[:], xt[:], ydt[:])
        tgarb = sq.tile([P, D], dt.bfloat16, tag="sqg", name="tgarb")
        nc.vector.tensor_scalar(
            out=tgarb[:], in0=prod[:], scalar1=0.0, scalar2=None,
            op0=Alu.add, op1=Alu.add, accum_out=xy[:],
        )
        sii = sm.tile([P, 1], dt.float32, tag="sii", name="sii")
        nc.vector.tensor_mul(sii[:], xy[:], rx[:])
        nc.vector.tensor_mul(sii[:], sii[:], ryd[:])
        rel = sm.tile([P, 1], dt.float32, tag="rel", name="rel")
        nc.vector.tensor_scalar_max(rel[:], sii[:], 0.0)
        dcol = XT * NJBLK + it
        nc.vector.tensor_add(colsums[:, dcol:dcol + 1], sii[:], rel[:])

    nc.sync.dma_start(out_ap[:, :], colsums[:])


_PROGRAM_CACHE = {}


def build_program():
    if "nc" in _PROGRAM_CACHE:
        return _PROGRAM_CACHE["nc"]
    nc = bacc.Bacc(
        "TRN2",
        target_bir_lowering=False,
        debug=False,
        enable_asserts=False,
    )
    x_ap = nc.dram_tensor("x", [M, D], dt.float32, kind="ExternalInput").ap()
    y_ap = nc.dram_tensor("y", [N, D], dt.float32, kind="ExternalInput").ap()
    yd_ap = nc.dram_tensor("ydiag", [M, D], dt.float32, kind="ExternalInput").ap()
    out_ap = nc.dram_tensor("out", [P, OUT_COLS], dt.float32, kind="ExternalOutput").ap()

    with tile.TileContext(nc) as tc:
        with ExitStack() as ctx:
            kernel_body(ctx, tc, out_ap, x_ap, y_ap, yd_ap)
    nc.compile()
    _PROGRAM_CACHE["nc"] = nc
    return nc


def make_in_maps(x: np.ndarray, y: np.ndarray):
    x = np.ascontiguousarray(np.asarray(x, dtype=np.float32))
    y = np.ascontiguousarray(np.asarray(y, dtype=np.float32))
    in_maps = []
    for c in range(NCORES):
        sl = slice(c * M, (c + 1) * M)
        in_maps.append({
            "x": x[sl],
            "y": y,
            "ydiag": y[sl],
        })
    return in_maps


def finish(outs: "list[np.ndarray]") -> np.float32:
    """Fold per-core [128, 136] partials into the scalar loss (float64)."""
    total = 0.0
    for o in outs:
        o = np.asarray(o, dtype=np.float64)
        main = o[:, :XT * NJBLK].sum()
        diag = o[:, XT * NJBLK:].sum()
        total += main + float(M) - diag
    return np.float32(total / (float(N) * float(N)))


def kernel(x: np.ndarray, y: np.ndarray):
    nc = build_program()
    in_maps = make_in_maps(x, y)
    res = bass_utils.run_bass_kernel_spmd(nc, in_maps, core_ids=list(range(NCORES)))
    outs = [r["out"] for r in res.results]
    loss = finish(outs)
    kernel.last_results = res
    return {"loss": loss}


kernel.last_results = None
